# revision 16
# baseline (speedup 1.0000x reference)
"""Trainium2 Bass kernel for a single-head AttentionBlock with residual.

Reference computation (per batch b):
    q = x @ Wq^T ; k = x @ Wk^T ; v = x @ Wv^T        (bq/bk zero per spec)
    s = (q @ k^T) / sqrt(D)                            [S, S]
    s = where(mask[b] == 0 (keys), -1e10, s)
    a = softmax(s, axis=-1)
    out = x + (a @ v) @ Wo^T + (Wo bv + bo)

Algebraic restructure (exact):
  * scores = x_q @ (Wq^T Wk) @ x_k^T -- fold Wq into the K projection:
        ktil = x_k @ (Wk^T Wq)  =>  scores = x_q . ktil   (no Q projection)
  * (a @ v) @ Wo^T = a @ (x_k @ (Wo Wv)^T) -- fold Wo into the V projection:
        vtil = x_k @ (Wo Wv)^T  =>  out = x_q + a @ vtil  (no out projection)
  * masked keys contribute exactly 0 to softmax num/denom (exp(-1e10) == 0
    in fp32), so keys are host-compacted: only kept keys (mask==1) are
    shipped/projected, padded up to KT*128 with -30000-bias slots.

Sharding: 8 cores = 4 batches x 2 query-halves, no collectives. Each core
projects ktil/vtil for all kept keys of its batch (~1028-1044 here, padded
to 1152) and attends its 1024 queries. ~7 GFLOP/core of matmul.

All matmuls run in fp8 (e4m3, TRN flavor: max +-240) with
perf_mode=DoubleRow: 256-row virtual contraction, 0.5 cycles per output
element. Weights are host-scaled x32 so their entries are ~N(0,1) in fp8;
the 1/32 is removed in the PSUM evictions. 1/sqrt(D) is applied as the exp
activation scale; exp is additionally scaled by 1/16 (bias -ln16) so the
fp8 expt tile stays in e4m3 range. The softmax denominator comes from
ones-vector DoubleRow matmuls accumulated over key tiles, transposed to
per-partition scalars with tiny fp32 matmuls, and applied together with
the residual add in one DVE scalar_tensor_tensor per output chunk.

Softmax max-subtraction is skipped: scores are ~N(0,1), exp < ~200 fits
fp32 and the /16 keeps expt in fp8 range.

nonzero bq/bk (spec says zeros) or an all-masked batch trigger an exact
numpy fallback.
"""

import functools
from contextlib import ExitStack

import ml_dtypes
import numpy as np

import concourse.bass as bass
import concourse.tile as tile
from concourse import bacc, mybir
from concourse.bass_utils import run_bass_kernel_spmd

P = 128
NEG_BIAS = -30000.0
N_CORES = 8
WSCALE = 32.0        # weight tensors stored x32 so entries are ~N(0,1) in fp8
EXP_SCALE = 16.0     # exp stored /16 so expt stays in e4m3 range
NP_FP8 = ml_dtypes.float8_e4m3  # TRN float8e4: max normal +-240


def _chunks(total, size):
    return [(o, min(size, total - o)) for o in range(0, total, size)]


def build_program(D=1024, SQ=1024, KT=10, n_cores=8, dedup=True):
    """Build + compile the single-core Bass program (same program on all cores).

    KT: number of 128-row key tiles in the gathered key space.
    dedup=True: each pair member projects only its local half of the kept
    keys (KT//2 tiles) and the halves are exchanged with the paired core via
    2-member AllGathers through a DRAM bounce.
    """
    f32 = mybir.dt.float32
    f16 = mybir.dt.float16
    fp8 = mybir.dt.float8e4
    DR = mybir.MatmulPerfMode.DoubleRow
    DT = D // P          # contraction tiles over d (and d' / e)
    QT = SQ // P         # query row tiles
    KPAD = KT * P
    assert DT % 2 == 0
    if dedup:
        assert KT % 2 == 0
        KLOC = (KT // 2) * P     # local key slots per pair member
    else:
        KLOC = KPAD

    Exp = mybir.ActivationFunctionType.Exp
    mult = mybir.AluOpType.mult
    add = mybir.AluOpType.add

    nc = bacc.Bacc("TRN2", target_bir_lowering=False, debug=False,
                   num_devices=n_cores)

    xqt_d = nc.dram_tensor("xqt", [D, SQ], fp8, kind="ExternalInput")
    xkt_d = nc.dram_tensor("xkt", [D, KLOC], fp8, kind="ExternalInput")
    mt_d = nc.dram_tensor("mt", [D, D], fp8, kind="ExternalInput")   # (Wk^T Wq)*32
    wvo_d = nc.dram_tensor("wvo", [D, D], fp8, kind="ExternalInput")  # (Wo Wv)^T*32
    mb_d = nc.dram_tensor("mb", [P, KT], f32, kind="ExternalInput")
    hs_d = nc.dram_tensor("hs", [SQ, D], f16, kind="ExternalInput")
    out_d = nc.dram_tensor("out", [SQ, D], f32, kind="ExternalOutput")

    with tile.TileContext(nc) as tc, ExitStack() as ctx:
        sb = ctx.enter_context(tc.tile_pool(name="sb", bufs=1))
        outp = ctx.enter_context(tc.tile_pool(name="outs", bufs=2))
        con = ctx.enter_context(tc.tile_pool(name="const", bufs=1))
        pp = ctx.enter_context(tc.tile_pool(name="pp", bufs=5, space="PSUM"))
        rsp = ctx.enter_context(tc.tile_pool(name="rsp", bufs=1, space="PSUM"))

        # ---- PE warmup during the initial DMA wait (HAM ramp) ----
        # tiny matmuls keep the PE activity monitor busy so the clock gate
        # is at 8/8 by the time the first real matmul issues; they end
        # before the k-proj inputs can possibly have landed.
        ones1h = con.tile([1, 1], f16)
        nc.gpsimd.memset(ones1h[:], 1.0)
        warm_in = con.tile([1, 64], f16)
        nc.gpsimd.memset(warm_in[:], 0.0)
        warm_ps = pp.tile([P, 512], f32, tag="pp")
        N_WARM = 80 if dedup else 140
        for i in range(N_WARM):
            nc.tensor.matmul(warm_ps[:1, :64], ones1h[:], warm_in[:],
                             start=(i == 0), stop=(i == N_WARM - 1))
        warm_out = con.tile([1, 64], f32)
        nc.vector.tensor_copy(warm_out[:], warm_ps[:1, :64])

        # ---- constants ----
        mb = con.tile([P, KT], f32)
        nc.gpsimd.dma_start(mb[:], mb_d.ap())
        ones1 = con.tile([1, 1], f32)
        nc.gpsimd.memset(ones1[:], 1.0)
        # fp8 ones for the row-sum matmuls; [P, 2, 16] so the pair-dim
        # stride is 16B (DoubleRow weight-AP steps must be 16B-aligned)
        onesk = con.tile([P, 2, 16], fp8)
        nc.gpsimd.memset(onesk[:], 1.0)

        if dedup:
            # CC-core warmup: the first collective of a kernel pays a
            # boot-once dispatch cost on the collectives core. Issue a tiny
            # dummy AllGather immediately so the real exchanges dispatch as
            # soon as their inputs are staged.
            pairs = [[2 * b, 2 * b + 1] for b in range(n_cores // 2)]
            dram = ctx.enter_context(
                tc.tile_pool(name="dram", bufs=1, space="DRAM"))
            ccw_in_d = dram.tile([P, 16], f32, tag="ccwi", name="ccw_in")
            ccw_out_d = dram.tile([2 * P, 16], f32, tag="ccwo", name="ccw_out")
            ccw_sb = con.tile([P, 16], f32)
            nc.gpsimd.memset(ccw_sb[:], 0.0)
            nc.sync.dma_start(ccw_in_d[:], ccw_sb[:])
            nc.gpsimd.collective_compute(
                "AllGather", mybir.AluOpType.bypass, replica_groups=pairs,
                ins=[ccw_in_d[:].opt()], outs=[ccw_out_d[:].opt()],
            )
            kt_loc_d = dram.tile([D, KLOC], fp8, tag="ktl", name="kt_loc")
            kt_g_d = dram.tile([2 * D, KLOC], fp8, tag="ktg", name="kt_g")
            v_loc_d = dram.tile([KLOC, D], fp8, tag="vl", name="v_loc")
            v_g_d = dram.tile([2 * KLOC, D], fp8, tag="vg", name="v_g")

        # ---- DMA loads (gpsimd / sync / scalar are the DMA queues).
        # The k-proj needs all of mt + xkt before its first accumulation
        # group can close: spread those bytes evenly across the 3 queues.
        mt_sb = sb.tile([P, DT, D], fp8)
        xkt_sb = sb.tile([P, DT, KLOC], fp8)
        wvo_sb = sb.tile([P, DT, D], fp8)
        xqt_sb = sb.tile([P, DT, SQ], fp8)
        hs_sb = sb.tile([P, QT, D], f16)

        mt_v = mt_d.ap().rearrange("(t p) e -> p t e", p=P)
        xkt_v = xkt_d.ap().rearrange("(t p) k -> p t k", p=P)
        wvo_v = wvo_d.ap().rearrange("(t p) e -> p t e", p=P)
        xqt_v = xqt_d.ap().rearrange("(t p) q -> p t q", p=P)
        hs_v = hs_d.ap().rearrange("(t p) f -> p t f", p=P)

        _ld = [nc.gpsimd, nc.sync, nc.scalar]
        H = DT // 2
        nc.gpsimd.dma_start(mt_sb[:, 0:H, :], mt_v[:, 0:H, :])
        nc.sync.dma_start(mt_sb[:, H:, :], mt_v[:, H:, :])
        nc.scalar.dma_start(xkt_sb[:, 0:H, :], xkt_v[:, 0:H, :])
        nc.sync.dma_start(xkt_sb[:, H:, :], xkt_v[:, H:, :])
        nc.gpsimd.dma_start(wvo_sb[:, 0:H, :], wvo_v[:, 0:H, :])
        nc.sync.dma_start(wvo_sb[:, H:, :], wvo_v[:, H:, :])
        if not dedup:
            # with dedup these are emitted after the collective stage-outs
            # so they don't park ahead of them in the queues
            nc.scalar.dma_start(xqt_sb[:, 0:DT // 2, :], xqt_v[:, 0:DT // 2, :])
            nc.gpsimd.dma_start(xqt_sb[:, DT // 2:, :], xqt_v[:, DT // 2:, :])
            nc.gpsimd.dma_start(hs_sb[:, 0:QT // 2, :], hs_v[:, 0:QT // 2, :])
            nc.sync.dma_start(hs_sb[:, QT // 2:, :], hs_v[:, QT // 2:, :])

        ktil = sb.tile([P, DT, KPAD], fp8)   # ktil^T: [d'-part, d'-tile, k]
        vtil = sb.tile([P, KT, D], fp8)      # vtil:   [k-part, k-tile, f]
        expt = sb.tile([P, KT, SQ], fp8)     # exp(scores)^T/16: [k-part, k-tile, q]
        if dedup:
            ktl_s = sb.tile([P, DT, KLOC], fp8)   # local k-proj staging
            vtl_s = sb.tile([P, KT // 2, D], fp8)  # local v-proj staging
        else:
            ktl_s, vtl_s = ktil, vtil

        # only DVE and ACT can read PSUM (GPSIMD cannot)
        _ev = [nc.vector, nc.scalar]
        evi = 0

        def evict(dst, src_ps):
            nonlocal evi
            e = _ev[evi % 2]
            evi += 1
            if e is nc.scalar:
                e.mul(dst, src_ps, 1.0 / WSCALE)
            else:
                e.tensor_scalar_mul(dst, src_ps, 1.0 / WSCALE)

        # ---- ktil_loc = ((Wk^T Wq) @ x_k^T)  [d', k_loc], DoubleRow over d ----
        kchunks = _chunks(KLOC, 512)
        for et in range(DT):
            pss = [pp.tile([P, 512], f32, tag="pp", name=f"ps_k{et}_{i}")
                   for i in range(len(kchunks))]
            for dp in range(DT // 2):
                lhsT = mt_sb[:, 2 * dp:2 * dp + 2, et * P:(et + 1) * P]
                for ci, (ko, kn) in enumerate(kchunks):
                    nc.tensor.matmul(
                        pss[ci][:, :kn], lhsT,
                        xkt_sb[:, 2 * dp:2 * dp + 2, ko:ko + kn],
                        start=(dp == 0), stop=(dp == DT // 2 - 1),
                        perf_mode=DR)
            for ci, (ko, kn) in enumerate(kchunks):
                evict(ktl_s[:, et, ko:ko + kn], pss[ci][:, :kn])

        if dedup:
            ktl_v = kt_loc_d[:].rearrange("(t p) k -> p t k", p=P)
            nc.sync.dma_start(ktl_v[:, 0:DT // 2, :], ktl_s[:, 0:DT // 2, :])
            nc.scalar.dma_start(ktl_v[:, DT // 2:, :], ktl_s[:, DT // 2:, :])
            nc.gpsimd.collective_compute(
                "AllGather", mybir.AluOpType.bypass, replica_groups=pairs,
                ins=[kt_loc_d[:].opt()], outs=[kt_g_d[:].opt()],
            )
            # scores inputs: emitted here so they queue behind the stage-outs
            nc.sync.dma_start(xqt_sb[:, 0:DT // 2, :], xqt_v[:, 0:DT // 2, :])
            nc.scalar.dma_start(xqt_sb[:, DT // 2:, :], xqt_v[:, DT // 2:, :])

        # ---- vtil_loc = x_k @ (Wo Wv)^T  [k_loc, f], DoubleRow over d ----
        fchunks = _chunks(D, 512)
        for kt in range(KLOC // P):
            pss = [pp.tile([P, 512], f32, tag="pp", name=f"ps_v{kt}_{i}")
                   for i in range(len(fchunks))]
            for dp in range(DT // 2):
                lhsT = xkt_sb[:, 2 * dp:2 * dp + 2, kt * P:(kt + 1) * P]
                for ci, (fo, fn) in enumerate(fchunks):
                    nc.tensor.matmul(
                        pss[ci][:, :fn], lhsT,
                        wvo_sb[:, 2 * dp:2 * dp + 2, fo:fo + fn],
                        start=(dp == 0), stop=(dp == DT // 2 - 1),
                        perf_mode=DR)
            for ci, (fo, fn) in enumerate(fchunks):
                evict(vtl_s[:, kt, fo:fo + fn], pss[ci][:, :fn])

        if dedup:
            vl_v = v_loc_d[:].rearrange("(t p) e -> p t e", p=P)
            VH = (KT // 2 + 1) // 2  # first stage-out block (of KT//2 tiles)
            nc.sync.dma_start(vl_v[:, 0:VH, :], vtl_s[:, 0:VH, :])
            nc.scalar.dma_start(vl_v[:, VH:, :], vtl_s[:, VH:, :])
            nc.gpsimd.collective_compute(
                "AllGather", mybir.AluOpType.bypass, replica_groups=pairs,
                ins=[v_loc_d[:].opt()], outs=[v_g_d[:].opt()],
            )
            # gather-ins: both members' blocks into the global tiles
            for m in range(2):
                _ld[m].dma_start(
                    ktil[:, :, m * KLOC:(m + 1) * KLOC],
                    kt_g_d[:][m * D:(m + 1) * D, :].rearrange(
                        "(t p) k -> p t k", p=P))
            vg_v = v_g_d[:].rearrange("(t p) f -> p t f", p=P)
            nc.scalar.dma_start(vtil[:, 0:KT // 2, :], vg_v[:, 0:KT // 2, :])
            nc.gpsimd.dma_start(vtil[:, KT // 2:, :], vg_v[:, KT // 2:, :])
            # residual rows last: needed only by the output stage
            nc.gpsimd.dma_start(hs_sb[:, 0:QT // 2, :], hs_v[:, 0:QT // 2, :])
            nc.sync.dma_start(hs_sb[:, QT // 2:, :], hs_v[:, QT // 2:, :])

        # ---- scores + softmax + A@V in two query halves so the first
        # half's output DMA drains behind the second half's compute ----
        rsum_sb = con.tile([1, SQ], f32)
        rinv = con.tile([P, QT], f32)
        out_v = out_d.ap().rearrange("(t p) f -> t p f", p=P)
        out_engs = [nc.sync, nc.gpsimd, nc.scalar]
        QH = QT // 2          # q tiles per half
        NH = QH * P           # queries per half

        def av_mms(qt):
            """A@V DoubleRow matmuls for one query tile; returns psum banks."""
            pss = [pp.tile([P, 512], f32, tag="pp", name=f"ps_o{qt}_{i}")
                   for i in range(len(fchunks))]
            for ktp in range(KT // 2):
                lhsT = expt[:, 2 * ktp:2 * ktp + 2, qt * P:(qt + 1) * P]
                for ci, (fo, fn) in enumerate(fchunks):
                    nc.tensor.matmul(
                        pss[ci][:, :fn], lhsT,
                        vtil[:, 2 * ktp:2 * ktp + 2, fo:fo + fn],
                        start=(ktp == 0),
                        stop=(ktp == KT // 2 - 1 and KT % 2 == 0),
                        perf_mode=DR)
            if KT % 2 == 1:
                lhsT = expt[:, KT - 1, qt * P:(qt + 1) * P]
                for ci, (fo, fn) in enumerate(fchunks):
                    nc.tensor.matmul(
                        pss[ci][:, :fn], lhsT,
                        vtil[:, KT - 1, fo:fo + fn],
                        start=(KT == 1), stop=True)
            return pss

        def stt_out(qt, pss):
            """normalize + residual + per-chunk output DMAs for one q tile."""
            outt = outp.tile([P, D], f32, tag="outt", name=f"outt{qt}")
            # chunk 0 on DVE; chunk 1 split ACT (psum*rinv) + GPSIMD (+hs)
            (fo, fn) = fchunks[0]
            nc.vector.scalar_tensor_tensor(
                outt[:, fo:fo + fn], pss[0][:, :fn], rinv[:, qt:qt + 1],
                hs_sb[:, qt, fo:fo + fn], op0=mult, op1=add)
            out_engs[(2 * qt) % 3].dma_start(
                out_v[qt][:, fo:fo + fn], outt[:, fo:fo + fn])
            if len(fchunks) > 1:
                (fo, fn) = fchunks[1]
                tmp = outp.tile([P, 512], f32, tag="tmp", name=f"tmp{qt}")
                nc.scalar.activation(
                    tmp[:, :fn], pss[1][:, :fn],
                    mybir.ActivationFunctionType.Copy,
                    scale=rinv[:, qt:qt + 1])
                nc.gpsimd.tensor_add(outt[:, fo:fo + fn], tmp[:, :fn],
                                     hs_sb[:, qt, fo:fo + fn])
                out_engs[(2 * qt + 1) % 3].dma_start(
                    out_v[qt][:, fo:fo + fn], outt[:, fo:fo + fn])

        for qh in range(2):
            qo = qh * NH
            # scores^T + exp for this half: expt = exp(s/sqrt(D) - ln16 + mb)
            rs = rsp.tile([1, 512], f32, tag="rs", name=f"rs{qh}")
            for kt in range(KT):
                ps = pp.tile([P, 512], f32, tag="pp", name=f"ps_s{qh}_{kt}")
                for ep in range(DT // 2):
                    nc.tensor.matmul(
                        ps[:, :NH], ktil[:, 2 * ep:2 * ep + 2, kt * P:(kt + 1) * P],
                        xqt_sb[:, 2 * ep:2 * ep + 2, qo:qo + NH],
                        start=(ep == 0), stop=(ep == DT // 2 - 1),
                        perf_mode=DR)
                nc.scalar.activation(
                    expt[:, kt, qo:qo + NH], ps[:, :NH], Exp,
                    bias=mb[:, kt:kt + 1], scale=float(D) ** -0.5)
                # softmax denominator: rs[1, q] += ones.T @ expt (kt pairs)
                if kt % 2 == 1:
                    nc.tensor.matmul(
                        rs[:1, :NH], onesk[:, :, 0:1],
                        expt[:, kt - 1:kt + 1, qo:qo + NH],
                        start=(kt == 1), stop=(kt == KT - 1),
                        perf_mode=DR)
            if KT % 2 == 1:
                nc.tensor.matmul(
                    rs[:1, :NH], onesk[:, 0, 0:1],
                    expt[:, KT - 1, qo:qo + NH],
                    start=(KT == 1), stop=True)

            # A@V matmuls of the first q tile overlap the rsum transpose
            pss0 = av_mms(qh * QH)

            # 1/rsum -> per-partition scalars: [1, NH] -> [P, QH]
            nc.scalar.copy(rsum_sb[:, qo:qo + NH], rs[:1, :NH])
            rsT = rsp.tile([P, QH], f32, tag="rsT", name=f"rsT{qh}")
            for t in range(QH):
                nc.tensor.matmul(
                    rsT[:, t:t + 1],
                    rsum_sb[:, qo + t * P:qo + (t + 1) * P], ones1[:],
                    start=(t == 0), stop=(t == QH - 1))
            nc.vector.reciprocal(rinv[:, qh * QH:(qh + 1) * QH], rsT[:])

            stt_out(qh * QH, pss0)
            for qt in range(qh * QH + 1, (qh + 1) * QH):
                stt_out(qt, av_mms(qt))

    nc.compile()
    return nc


DEDUP = False


@functools.lru_cache(maxsize=4)
def _get_program(D, SQ, KT, dedup=DEDUP):
    return build_program(D, SQ, KT, dedup=dedup)


def _numpy_reference(hidden_states, mask, Wq, bq, Wk, bk, Wv, bv, Wo, bo):
    """Exact fallback (used only for inputs outside the spec envelope)."""
    x = hidden_states.astype(np.float64)
    q = x @ Wq.T.astype(np.float64) + bq
    k = x @ Wk.T.astype(np.float64) + bk
    v = x @ Wv.T.astype(np.float64) + bv
    s = np.einsum("bqd,bkd->bqk", q, k) / np.sqrt(x.shape[-1])
    s = np.where(mask[:, None, :] == 0, -1e10, s)
    s -= s.max(axis=-1, keepdims=True)
    e = np.exp(s)
    a = e / e.sum(axis=-1, keepdims=True)
    hid = np.einsum("bqk,bkd->bqd", a, v)
    out = x + hid @ Wo.T.astype(np.float64) + bo
    return out.astype(np.float32)


def _fp8(a):
    return np.ascontiguousarray(
        np.clip(a, -240.0, 240.0).astype(NP_FP8))


def pick_kt(mask, dedup=DEDUP):
    nb = (np.asarray(mask) != 0).sum(axis=1)
    if dedup:
        # per-member local slots must cover ceil(nb/2); gathered = 2 halves
        kth = (int(nb.max() + 1) // 2 + P - 1) // P
        return 2 * kth, nb
    return (int(nb.max()) + P - 1) // P, nb


def make_in_maps(hidden_states, mask, Wq, bq, Wk, bk, Wv, bv, Wo, bo, KT,
                 dedup=DEDUP):
    hs = np.asarray(hidden_states, dtype=np.float32)
    mask = np.asarray(mask)
    B, S, D = hs.shape
    SQ = S // 2
    KPAD = KT * P

    Wq64 = np.asarray(Wq, np.float64)
    Wk64 = np.asarray(Wk, np.float64)
    Wv64 = np.asarray(Wv, np.float64)
    Wo64 = np.asarray(Wo, np.float64)
    # scores = x_q @ (Wq^T Wk) @ x_k^T ; ktil-proj lhsT[d, d'] = (Wk^T Wq)[d, d']
    mt_h = _fp8(Wk64.T @ Wq64 * WSCALE)
    # out = a @ (x_k @ (Wo Wv)^T) ; vtil-proj rhs[d, f] = (Wo Wv)^T[d, f]
    wvo_h = _fp8((Wo64 @ Wv64).T * WSCALE)
    # v-bias and o-bias act as a constant shift after the output projection
    extra = (np.asarray(Wo, np.float32) @ np.asarray(bv, np.float32)
             + np.asarray(bo, np.float32))

    in_maps = []
    for c in range(N_CORES):
        b, h = divmod(c, 2)
        xb = hs[b]
        keep = np.nonzero(mask[b] != 0)[0]
        nb = len(keep)
        if dedup:
            # member h projects half the kept keys; gathered key space is
            # [member0 block | member1 block], KLOC slots each
            KLOC = KPAD // 2
            n0 = (nb + 1) // 2
            loc = keep[:n0] if h == 0 else keep[n0:]
            xk = np.zeros((KLOC, D), np.float32)
            xk[:len(loc)] = xb[loc]
            mbv = np.full(KPAD, NEG_BIAS, np.float32)
            mbv[:n0] = -np.log(EXP_SCALE)
            mbv[KLOC:KLOC + nb - n0] = -np.log(EXP_SCALE)
        else:
            xk = np.zeros((KPAD, D), np.float32)
            xk[:nb] = xb[keep]
            mbv = np.full(KPAD, NEG_BIAS, np.float32)
            mbv[:nb] = -np.log(EXP_SCALE)
        xq = xb[h * SQ:(h + 1) * SQ]
        m = dict(
            xqt=_fp8(xq.T),
            xkt=_fp8(xk.T),
            mt=mt_h, wvo=wvo_h,
            mb=np.ascontiguousarray(mbv.reshape(KT, P).T),
            hs=np.ascontiguousarray((xq + extra[None, :]).astype(np.float16)),
        )
        in_maps.append(m)
    return in_maps


def assemble_output(results, B, S, D):
    SQ = S // 2
    out = np.empty((B, S, D), np.float32)
    for c in range(N_CORES):
        b, h = divmod(c, 2)
        out[b, h * SQ:(h + 1) * SQ, :] = results[c]["out"]
    return out


def kernel(hidden_states, mask, Wq, bq, Wk, bk, Wv, bv, Wo, bo):
    hs = np.asarray(hidden_states, dtype=np.float32)
    B, S, D = hs.shape
    args = dict(hidden_states=hs, mask=np.asarray(mask),
                Wq=np.asarray(Wq, np.float32), bq=np.asarray(bq, np.float32),
                Wk=np.asarray(Wk, np.float32), bk=np.asarray(bk, np.float32),
                Wv=np.asarray(Wv, np.float32), bv=np.asarray(bv, np.float32),
                Wo=np.asarray(Wo, np.float32), bo=np.asarray(bo, np.float32))
    KT, nb = pick_kt(args["mask"])
    if (np.any(args["bq"]) or np.any(args["bk"]) or nb.min() == 0
            or B * 2 != N_CORES or S % 256 or D % 256 or D < 512):
        return _numpy_reference(**args)

    nc = _get_program(D, S // 2, KT)
    in_maps = make_in_maps(**args, KT=KT)
    res = run_bass_kernel_spmd(nc, in_maps, core_ids=list(range(N_CORES)))
    return assemble_output(res.results, B, S, D)


if __name__ == "__main__":
    rng = np.random.default_rng(0)
    B, S, D = 4, 2048, 1024
    ins = dict(
        hidden_states=rng.standard_normal((B, S, D), np.float32),
        mask=rng.integers(0, 2, (B, S)).astype(np.int32),
        Wq=rng.standard_normal((D, D), np.float32) / np.sqrt(D),
        bq=np.zeros(D, np.float32),
        Wk=rng.standard_normal((D, D), np.float32) / np.sqrt(D),
        bk=np.zeros(D, np.float32),
        Wv=rng.standard_normal((D, D), np.float32) / np.sqrt(D),
        bv=np.zeros(D, np.float32),
        Wo=rng.standard_normal((D, D), np.float32) / np.sqrt(D),
        bo=np.zeros(D, np.float32),
    )
    out = kernel(**ins)
    ref = _numpy_reference(**ins)
    err = np.max(np.abs(out - ref)) / np.max(np.abs(ref))
    print("rel err vs numpy:", err)


# revision 17
# speedup vs baseline: 1.0586x; 1.0586x over previous
"""Trainium2 Bass kernel for a single-head AttentionBlock with residual.

Reference computation (per batch b):
    q = x @ Wq^T ; k = x @ Wk^T ; v = x @ Wv^T        (bq/bk zero per spec)
    s = (q @ k^T) / sqrt(D)                            [S, S]
    s = where(mask[b] == 0 (keys), -1e10, s)
    a = softmax(s, axis=-1)
    out = x + (a @ v) @ Wo^T + (Wo bv + bo)

Algebraic restructure (exact):
  * scores = x_q @ (Wq^T Wk) @ x_k^T -- fold Wq into the K projection:
        ktil = x_k @ (Wk^T Wq)  =>  scores = x_q . ktil   (no Q projection)
  * (a @ v) @ Wo^T = a @ (x_k @ (Wo Wv)^T) -- fold Wo into the V projection:
        vtil = x_k @ (Wo Wv)^T  =>  out = x_q + a @ vtil  (no out projection)
  * masked keys contribute exactly 0 to softmax num/denom (exp(-1e10) == 0
    in fp32), so keys are host-compacted: only kept keys (mask==1) are
    shipped/projected, padded up to KT*128 with -30000-bias slots.

Sharding: 8 cores = 4 batches x 2 query-halves, no collectives. Each core
projects ktil/vtil for all kept keys of its batch (~1028-1044 here, padded
to 1152) and attends its 1024 queries. ~7 GFLOP/core of matmul.

All matmuls run in fp8 (e4m3, TRN flavor: max +-240) with
perf_mode=DoubleRow: 256-row virtual contraction, 0.5 cycles per output
element. Weights are host-scaled x32 so their entries are ~N(0,1) in fp8;
the 1/32 is removed in the PSUM evictions. 1/sqrt(D) is applied as the exp
activation scale; exp is additionally scaled by 1/16 (bias -ln16) so the
fp8 expt tile stays in e4m3 range. The softmax denominator comes from
ones-vector DoubleRow matmuls accumulated over key tiles, transposed to
per-partition scalars with tiny fp32 matmuls, and applied together with
the residual add in one DVE scalar_tensor_tensor per output chunk.

Softmax max-subtraction is skipped: scores are ~N(0,1), exp < ~200 fits
fp32 and the /16 keeps expt in fp8 range.

nonzero bq/bk (spec says zeros) or an all-masked batch trigger an exact
numpy fallback.
"""

import functools
from contextlib import ExitStack

import ml_dtypes
import numpy as np

import concourse.bass as bass
import concourse.tile as tile
from concourse import bacc, mybir
from concourse.bass_utils import run_bass_kernel_spmd

P = 128
NEG_BIAS = -30000.0
N_CORES = 8
WSCALE = 32.0        # weight tensors stored x32 so entries are ~N(0,1) in fp8
EXP_SCALE = 16.0     # exp stored /16 so expt stays in e4m3 range
NP_FP8 = ml_dtypes.float8_e4m3  # TRN float8e4: max normal +-240


def _chunks(total, size):
    return [(o, min(size, total - o)) for o in range(0, total, size)]


def build_program(D=1024, SQ=1024, KT=9, n_cores=8, dedup=False):
    """Build + compile the single-core Bass program (same program on all cores).

    KT: number of 128-row key tiles (kept keys padded to KT*128).
    """
    f32 = mybir.dt.float32
    f16 = mybir.dt.float16
    fp8 = mybir.dt.float8e4
    DR = mybir.MatmulPerfMode.DoubleRow
    DT = D // P          # contraction tiles over d (and d' / e)
    QT = SQ // P         # query row tiles
    KPAD = KT * P
    assert DT % 2 == 0

    Exp = mybir.ActivationFunctionType.Exp
    mult = mybir.AluOpType.mult
    add = mybir.AluOpType.add

    nc = bacc.Bacc("TRN2", target_bir_lowering=False, debug=False,
                   num_devices=n_cores)

    xqt_d = nc.dram_tensor("xqt", [D, SQ], fp8, kind="ExternalInput")
    xkt_d = nc.dram_tensor("xkt", [D, KPAD], fp8, kind="ExternalInput")
    mt_d = nc.dram_tensor("mt", [D, D], fp8, kind="ExternalInput")   # (Wk^T Wq)*32
    wvo_d = nc.dram_tensor("wvo", [D, D], fp8, kind="ExternalInput")  # (Wo Wv)^T*32
    mb_d = nc.dram_tensor("mb", [P, KT], f32, kind="ExternalInput")
    hs_d = nc.dram_tensor("hs", [SQ, D], f16, kind="ExternalInput")
    out_d = nc.dram_tensor("out", [SQ, D], f32, kind="ExternalOutput")

    with tile.TileContext(nc) as tc, ExitStack() as ctx:
        sb = ctx.enter_context(tc.tile_pool(name="sb", bufs=1))
        outp = ctx.enter_context(tc.tile_pool(name="outs", bufs=2))
        con = ctx.enter_context(tc.tile_pool(name="const", bufs=1))
        pp = ctx.enter_context(tc.tile_pool(name="pp", bufs=5, space="PSUM"))
        rsp = ctx.enter_context(tc.tile_pool(name="rsp", bufs=1, space="PSUM"))

        # ---- PE warmup during the initial DMA wait (HAM ramp) ----
        ones1h = con.tile([1, 1], f16)
        nc.gpsimd.memset(ones1h[:], 1.0)
        warm_in = con.tile([1, 256], f16)
        nc.gpsimd.memset(warm_in[:], 0.0)
        warm_ps = pp.tile([P, 512], f32, tag="pp")
        N_WARM = 16
        for i in range(N_WARM):
            nc.tensor.matmul(warm_ps[:1, :256], ones1h[:], warm_in[:],
                             start=(i == 0), stop=(i == N_WARM - 1))
        warm_out = con.tile([1, 256], f32)
        nc.vector.tensor_copy(warm_out[:], warm_ps[:1, :256])

        # ---- constants ----
        mb = con.tile([P, KT], f32)
        nc.gpsimd.dma_start(mb[:], mb_d.ap())
        ones1 = con.tile([1, 1], f32)
        nc.gpsimd.memset(ones1[:], 1.0)
        # fp8 ones for the row-sum matmuls; [P, 2, 16] so the pair-dim
        # stride is 16B (DoubleRow weight-AP steps must be 16B-aligned)
        onesk = con.tile([P, 2, 16], fp8)
        nc.gpsimd.memset(onesk[:], 1.0)

        # ---- DMA loads: first-needed first, split across the 3 queues ----
        _engs = [nc.gpsimd, nc.sync, nc.scalar]

        mt_sb = sb.tile([P, DT, D], fp8)
        xkt_sb = sb.tile([P, DT, KPAD], fp8)
        wvo_sb = sb.tile([P, DT, D], fp8)
        xqt_sb = sb.tile([P, DT, SQ], fp8)
        hs_sb = sb.tile([P, QT, D], f16)

        mt_v = mt_d.ap().rearrange("(t p) e -> p t e", p=P)
        xkt_v = xkt_d.ap().rearrange("(t p) k -> p t k", p=P)
        wvo_v = wvo_d.ap().rearrange("(t p) e -> p t e", p=P)
        xqt_v = xqt_d.ap().rearrange("(t p) q -> p t q", p=P)
        hs_v = hs_d.ap().rearrange("(t p) f -> p t f", p=P)

        ei = 0
        # mt + xkt per contraction pair, interleaved so the first k-proj
        # accumulation group can start as soon as pair 0 lands
        for dp in range(DT // 2):
            sl = slice(2 * dp, 2 * dp + 2)
            _engs[ei % 3].dma_start(mt_sb[:, sl, :], mt_v[:, sl, :]); ei += 1
            _engs[ei % 3].dma_start(xkt_sb[:, sl, :], xkt_v[:, sl, :]); ei += 1
        for dp in range(DT // 2):
            sl = slice(2 * dp, 2 * dp + 2)
            _engs[ei % 3].dma_start(wvo_sb[:, sl, :], wvo_v[:, sl, :]); ei += 1
        for dp in range(DT // 2):
            sl = slice(2 * dp, 2 * dp + 2)
            _engs[ei % 3].dma_start(xqt_sb[:, sl, :], xqt_v[:, sl, :]); ei += 1
        for hh in range(2):
            sl = slice(hh * (QT // 2), (hh + 1) * (QT // 2))
            _engs[ei % 3].dma_start(hs_sb[:, sl, :], hs_v[:, sl, :]); ei += 1

        ktil = sb.tile([P, DT, KPAD], fp8)   # ktil^T: [d'-part, d'-tile, k]
        vtil = sb.tile([P, KT, D], fp8)      # vtil:   [k-part, k-tile, f]
        expt = sb.tile([P, KT, SQ], fp8)     # exp(scores)^T/16: [k-part, k-tile, q]

        # only DVE and ACT can read PSUM (GPSIMD cannot)
        _ev = [nc.vector, nc.scalar]
        evi = 0

        def evict(dst, src_ps):
            nonlocal evi
            e = _ev[evi % 2]
            evi += 1
            if e is nc.scalar:
                e.mul(dst, src_ps, 1.0 / WSCALE)
            else:
                e.tensor_scalar_mul(dst, src_ps, 1.0 / WSCALE)

        # ---- ktil = ((Wk^T Wq) @ x_k^T)  [d', k], DoubleRow over d ----
        kchunks = _chunks(KPAD, 512)
        for et in range(DT):
            pss = [pp.tile([P, 512], f32, tag="pp", name=f"ps_k{et}_{i}")
                   for i in range(len(kchunks))]
            for dp in range(DT // 2):
                lhsT = mt_sb[:, 2 * dp:2 * dp + 2, et * P:(et + 1) * P]
                for ci, (ko, kn) in enumerate(kchunks):
                    nc.tensor.matmul(
                        pss[ci][:, :kn], lhsT,
                        xkt_sb[:, 2 * dp:2 * dp + 2, ko:ko + kn],
                        start=(dp == 0), stop=(dp == DT // 2 - 1),
                        perf_mode=DR)
            for ci, (ko, kn) in enumerate(kchunks):
                evict(ktil[:, et, ko:ko + kn], pss[ci][:, :kn])

        # ---- vtil = x_k @ (Wo Wv)^T  [k, f], DoubleRow over d ----
        fchunks = _chunks(D, 512)
        for kt in range(KT):
            pss = [pp.tile([P, 512], f32, tag="pp", name=f"ps_v{kt}_{i}")
                   for i in range(len(fchunks))]
            for dp in range(DT // 2):
                lhsT = xkt_sb[:, 2 * dp:2 * dp + 2, kt * P:(kt + 1) * P]
                for ci, (fo, fn) in enumerate(fchunks):
                    nc.tensor.matmul(
                        pss[ci][:, :fn], lhsT,
                        wvo_sb[:, 2 * dp:2 * dp + 2, fo:fo + fn],
                        start=(dp == 0), stop=(dp == DT // 2 - 1),
                        perf_mode=DR)
            for ci, (fo, fn) in enumerate(fchunks):
                evict(vtil[:, kt, fo:fo + fn], pss[ci][:, :fn])

        # ---- scores^T + exp: expt = exp(s/sqrt(D) - ln16 + mb) ----
        # rs[1, q] += ones.T @ expt  (DoubleRow pairs of key tiles)
        qchunks = _chunks(SQ, 512)
        rss = [rsp.tile([1, 512], f32, tag=f"rs{ci}", name=f"rs{ci}")
               for ci in range(len(qchunks))]
        for kt in range(KT):
            pss = [pp.tile([P, 512], f32, tag="pp", name=f"ps_s{kt}_{i}")
                   for i in range(len(qchunks))]
            for ep in range(DT // 2):
                lhsT = ktil[:, 2 * ep:2 * ep + 2, kt * P:(kt + 1) * P]
                for ci, (qo, qn) in enumerate(qchunks):
                    nc.tensor.matmul(
                        pss[ci][:, :qn], lhsT,
                        xqt_sb[:, 2 * ep:2 * ep + 2, qo:qo + qn],
                        start=(ep == 0), stop=(ep == DT // 2 - 1),
                        perf_mode=DR)
            for ci, (qo, qn) in enumerate(qchunks):
                nc.scalar.activation(
                    expt[:, kt, qo:qo + qn], pss[ci][:, :qn], Exp,
                    bias=mb[:, kt:kt + 1], scale=float(D) ** -0.5)
            if kt % 2 == 1:
                for ci, (qo, qn) in enumerate(qchunks):
                    nc.tensor.matmul(
                        rss[ci][:, :qn], onesk[:, :, 0:1],
                        expt[:, kt - 1:kt + 1, qo:qo + qn],
                        start=(kt == 1), stop=(kt == KT - 1),
                        perf_mode=DR)
        if KT % 2 == 1:
            for ci, (qo, qn) in enumerate(qchunks):
                nc.tensor.matmul(
                    rss[ci][:, :qn], onesk[:, 0, 0:1],
                    expt[:, KT - 1, qo:qo + qn],
                    start=(KT == 1), stop=True)

        # ---- 1/rsum as per-partition scalars: [1, SQ] -> [P, QT] ----
        rsum_sb = con.tile([1, SQ], f32)
        for ci, (qo, qn) in enumerate(qchunks):
            nc.scalar.copy(rsum_sb[:, qo:qo + qn], rss[ci][:, :qn])
        rsT = rsp.tile([P, QT], f32, tag="rsT")
        for t in range(QT):
            nc.tensor.matmul(
                rsT[:, t:t + 1], rsum_sb[:, t * P:(t + 1) * P], ones1[:],
                start=(t == 0), stop=(t == QT - 1))
        rinv = con.tile([P, QT], f32)
        nc.vector.reciprocal(rinv[:], rsT[:])

        # ---- out[q, f] = (expt.T @ vtil) * rinv[q] + hs[q, f] ----
        out_v = out_d.ap().rearrange("(t p) f -> t p f", p=P)
        out_engs = [nc.sync, nc.scalar, nc.gpsimd]
        for qt in range(QT):
            pss = [pp.tile([P, 512], f32, tag="pp", name=f"ps_o{qt}_{i}")
                   for i in range(len(fchunks))]
            for ktp in range(KT // 2):
                lhsT = expt[:, 2 * ktp:2 * ktp + 2, qt * P:(qt + 1) * P]
                for ci, (fo, fn) in enumerate(fchunks):
                    nc.tensor.matmul(
                        pss[ci][:, :fn], lhsT,
                        vtil[:, 2 * ktp:2 * ktp + 2, fo:fo + fn],
                        start=(ktp == 0),
                        stop=(ktp == KT // 2 - 1 and KT % 2 == 0),
                        perf_mode=DR)
            if KT % 2 == 1:
                lhsT = expt[:, KT - 1, qt * P:(qt + 1) * P]
                for ci, (fo, fn) in enumerate(fchunks):
                    nc.tensor.matmul(
                        pss[ci][:, :fn], lhsT,
                        vtil[:, KT - 1, fo:fo + fn],
                        start=(KT == 1), stop=True)
            outt = outp.tile([P, D], f32, tag="outt", name=f"outt{qt}")
            for ci, (fo, fn) in enumerate(fchunks):
                nc.vector.scalar_tensor_tensor(
                    outt[:, fo:fo + fn], pss[ci][:, :fn], rinv[:, qt:qt + 1],
                    hs_sb[:, qt, fo:fo + fn], op0=mult, op1=add)
                out_engs[(qt * 2 + ci) % 3].dma_start(
                    out_v[qt][:, fo:fo + fn], outt[:, fo:fo + fn])

    nc.compile()
    return nc


DEDUP = False


@functools.lru_cache(maxsize=4)
def _get_program(D, SQ, KT, dedup=DEDUP):
    return build_program(D, SQ, KT, dedup=dedup)


def _numpy_reference(hidden_states, mask, Wq, bq, Wk, bk, Wv, bv, Wo, bo):
    """Exact fallback (used only for inputs outside the spec envelope)."""
    x = hidden_states.astype(np.float64)
    q = x @ Wq.T.astype(np.float64) + bq
    k = x @ Wk.T.astype(np.float64) + bk
    v = x @ Wv.T.astype(np.float64) + bv
    s = np.einsum("bqd,bkd->bqk", q, k) / np.sqrt(x.shape[-1])
    s = np.where(mask[:, None, :] == 0, -1e10, s)
    s -= s.max(axis=-1, keepdims=True)
    e = np.exp(s)
    a = e / e.sum(axis=-1, keepdims=True)
    hid = np.einsum("bqk,bkd->bqd", a, v)
    out = x + hid @ Wo.T.astype(np.float64) + bo
    return out.astype(np.float32)


def _fp8(a):
    return np.ascontiguousarray(
        np.clip(a, -240.0, 240.0).astype(NP_FP8))


def pick_kt(mask, dedup=DEDUP):
    nb = (np.asarray(mask) != 0).sum(axis=1)
    if dedup:
        # per-member local slots must cover ceil(nb/2); gathered = 2 halves
        kth = (int(nb.max() + 1) // 2 + P - 1) // P
        return 2 * kth, nb
    return (int(nb.max()) + P - 1) // P, nb


def make_in_maps(hidden_states, mask, Wq, bq, Wk, bk, Wv, bv, Wo, bo, KT,
                 dedup=DEDUP):
    hs = np.asarray(hidden_states, dtype=np.float32)
    mask = np.asarray(mask)
    B, S, D = hs.shape
    SQ = S // 2
    KPAD = KT * P

    Wq64 = np.asarray(Wq, np.float64)
    Wk64 = np.asarray(Wk, np.float64)
    Wv64 = np.asarray(Wv, np.float64)
    Wo64 = np.asarray(Wo, np.float64)
    # scores = x_q @ (Wq^T Wk) @ x_k^T ; ktil-proj lhsT[d, d'] = (Wk^T Wq)[d, d']
    mt_h = _fp8(Wk64.T @ Wq64 * WSCALE)
    # out = a @ (x_k @ (Wo Wv)^T) ; vtil-proj rhs[d, f] = (Wo Wv)^T[d, f]
    wvo_h = _fp8((Wo64 @ Wv64).T * WSCALE)
    # v-bias and o-bias act as a constant shift after the output projection
    extra = (np.asarray(Wo, np.float32) @ np.asarray(bv, np.float32)
             + np.asarray(bo, np.float32))

    in_maps = []
    for c in range(N_CORES):
        b, h = divmod(c, 2)
        xb = hs[b]
        keep = np.nonzero(mask[b] != 0)[0]
        nb = len(keep)
        if dedup:
            # member h projects half the kept keys; gathered key space is
            # [member0 block | member1 block], KLOC slots each
            KLOC = KPAD // 2
            n0 = (nb + 1) // 2
            loc = keep[:n0] if h == 0 else keep[n0:]
            xk = np.zeros((KLOC, D), np.float32)
            xk[:len(loc)] = xb[loc]
            mbv = np.full(KPAD, NEG_BIAS, np.float32)
            mbv[:n0] = -np.log(EXP_SCALE)
            mbv[KLOC:KLOC + nb - n0] = -np.log(EXP_SCALE)
        else:
            xk = np.zeros((KPAD, D), np.float32)
            xk[:nb] = xb[keep]
            mbv = np.full(KPAD, NEG_BIAS, np.float32)
            mbv[:nb] = -np.log(EXP_SCALE)
        xq = xb[h * SQ:(h + 1) * SQ]
        m = dict(
            xqt=_fp8(xq.T),
            xkt=_fp8(xk.T),
            mt=mt_h, wvo=wvo_h,
            mb=np.ascontiguousarray(mbv.reshape(KT, P).T),
            hs=np.ascontiguousarray((xq + extra[None, :]).astype(np.float16)),
        )
        in_maps.append(m)
    return in_maps


def assemble_output(results, B, S, D):
    SQ = S // 2
    out = np.empty((B, S, D), np.float32)
    for c in range(N_CORES):
        b, h = divmod(c, 2)
        out[b, h * SQ:(h + 1) * SQ, :] = results[c]["out"]
    return out


def kernel(hidden_states, mask, Wq, bq, Wk, bk, Wv, bv, Wo, bo):
    hs = np.asarray(hidden_states, dtype=np.float32)
    B, S, D = hs.shape
    args = dict(hidden_states=hs, mask=np.asarray(mask),
                Wq=np.asarray(Wq, np.float32), bq=np.asarray(bq, np.float32),
                Wk=np.asarray(Wk, np.float32), bk=np.asarray(bk, np.float32),
                Wv=np.asarray(Wv, np.float32), bv=np.asarray(bv, np.float32),
                Wo=np.asarray(Wo, np.float32), bo=np.asarray(bo, np.float32))
    KT, nb = pick_kt(args["mask"])
    if (np.any(args["bq"]) or np.any(args["bk"]) or nb.min() == 0
            or B * 2 != N_CORES or S % 256 or D % 256 or D < 512):
        return _numpy_reference(**args)

    nc = _get_program(D, S // 2, KT)
    in_maps = make_in_maps(**args, KT=KT)
    res = run_bass_kernel_spmd(nc, in_maps, core_ids=list(range(N_CORES)))
    return assemble_output(res.results, B, S, D)


if __name__ == "__main__":
    rng = np.random.default_rng(0)
    B, S, D = 4, 2048, 1024
    ins = dict(
        hidden_states=rng.standard_normal((B, S, D), np.float32),
        mask=rng.integers(0, 2, (B, S)).astype(np.int32),
        Wq=rng.standard_normal((D, D), np.float32) / np.sqrt(D),
        bq=np.zeros(D, np.float32),
        Wk=rng.standard_normal((D, D), np.float32) / np.sqrt(D),
        bk=np.zeros(D, np.float32),
        Wv=rng.standard_normal((D, D), np.float32) / np.sqrt(D),
        bv=np.zeros(D, np.float32),
        Wo=rng.standard_normal((D, D), np.float32) / np.sqrt(D),
        bo=np.zeros(D, np.float32),
    )
    out = kernel(**ins)
    ref = _numpy_reference(**ins)
    err = np.max(np.abs(out - ref)) / np.max(np.abs(ref))
    print("rel err vs numpy:", err)
